# revision 27
# baseline (speedup 1.0000x reference)
"""ChannelAttention kernel for Trainium2 (Bass/Tile), 8-core SPMD.

Reference (per sample b, xf = x[b] as [C=256, N=16384]):
    F  = W_f @ xf                      [50, N]
    S  = softmax(F @ xf^T, axis=C)     [50, 256]
    E  = S^T @ F ; out = W_beta @ E + xf

Algebraic restructure: out = (W_beta @ S^T) @ F + x = M @ F + x where
M = W_beta @ S^T is a tiny [256, 50] matrix computed once per sample after
softmax — the big E tensor is never materialized.

Sharding: 8 cores = 4 samples x 2 spatial halves (x[b][:, h*8192:(h+1)*8192]).
Cross-core coupling: AllGather of partial S^T within pairs
[[0,1],[2,3],[4,5],[6,7]] (51 KB each way) + local add.

Per-core dataflow (v2 — minimizes fp32 PE rows; PE cost is out_free x
cycles_per_row, fp32 matmul=4, fp32 transpose=2, fp32r/bf16 matmul=1 when
out_free>=256):
  phase 1 (per 128-col n-chunk, exact fp32 logit path):
      F^T[n,50]   = matmul(lhsT=x_ci[:,chunk], rhs=wft_ci)    ap=50
      x^T[n,128c] = PE-transpose(x_ci[:,chunk])               ap=128
      S^T[c,50]  += matmul(lhsT=x^T_ci, rhs=F^T)              ap=50
    (vs baseline's F/S with ap=512/256 at 4 cyc/row — ~2.3x fewer PE cycles)
  phase 1.5: partial S^T -> DRAM -> AllGather(pair) -> local add. DURING the
      collective the PE recomputes F = W_f x in fp32r (ap=512, full rate) for
      the phase-3 matmul — post-softmax path tolerates tf32 operand rounding.
  phase 2: transpose S^T -> S [50,256]; softmax over free axis; P=exp(S-mx);
      M^T = (P @ W_beta^T) scaled by 1/rowsum during ACT evacuation.
  phase 3: out = M F + x; residual adds alternate DVE/Pool so neither gates
      the store stream; 512 KiB store DMAs alternate SP/ACT HWDGE rings.

n_iters > 1 repeats the whole dataflow inside one NEFF — used by test.py to
measure per-iteration HW time by differencing (no NTFF under axon).
"""

import os
import numpy as np
from contextlib import ExitStack

import concourse.bass as bass
import concourse.tile as tile
from concourse import mybir
from concourse.bass_utils import run_bass_kernel_spmd
from concourse.masks import make_identity

B, C, O = 4, 256, 50
N = 128 * 128            # 16384 spatial positions
NCORES = 8
NH = N // 2              # 8192 per core
NT = 512                 # n-tile (4 sub-chunks of 128)
NSUB = 128
F32 = mybir.dt.float32
F32R = mybir.dt.float32r
BF16 = mybir.dt.bfloat16
ActF = mybir.ActivationFunctionType

_CACHE: dict = {}
last_results = None  # exposes BassKernelResults to test.py

# This walrus build rejects instructions carrying more than one embedded
# semaphore wait ("Too many sync wait commands" in setupSyncWait). After
# Tile finishes sem assignment, hoist excess waits onto InstNoOp
# instructions inserted before the offender on the same engine — engine
# program order makes the split semantically identical.
_MAX_WAITS = 1


def _split_multiwait(nc) -> int:
    n_nops = 0
    for fn in nc.m.functions:
        for blk in fn.blocks:
            out = []
            changed = False
            for inst in list(blk.instructions):
                si = inst.sync_info
                waits = list(si.on_wait) if si is not None and si.on_wait else []
                if len(waits) > _MAX_WAITS:
                    keep = waits[-_MAX_WAITS:]
                    hoist = waits[:-_MAX_WAITS]
                    for i in range(0, len(hoist), _MAX_WAITS):
                        nop = mybir.InstNoOp(name=f"I-waitnop-{n_nops}")
                        n_nops += 1
                        nop.engine = inst.engine
                        nop.sync_info = mybir.SyncInfo(
                            on_wait=hoist[i:i + _MAX_WAITS], on_update=[]
                        )
                        out.append(nop)
                    changed = True
                    inst.sync_info = mybir.SyncInfo(
                        on_wait=keep,
                        on_update=list(si.on_update) if si.on_update else [],
                    )
                out.append(inst)
            if changed:
                blk.instructions = out
    return n_nops


def _build_nc(fast: bool = False, n_iters: int = 1,
              skip_phase3: bool = False, skip_cc: bool = False) -> bass.Bass:
    """fast is accepted for test.py compatibility but ignored (the v2
    dataflow keeps the logit path in exact fp32 at full-rate-ish cost).
    skip_* flags build ablated variants for phase-isolation timing."""
    nc = bass.Bass(num_devices=NCORES)

    xs = nc.dram_tensor("xs", [2, 128, NH], F32R, kind="ExternalInput")
    wft = nc.dram_tensor("wft", [2, 128, O], F32R, kind="ExternalInput")
    wbt = nc.dram_tensor("wbt", [2, 128, C], F32R, kind="ExternalInput")
    out = nc.dram_tensor("out", [2, 128, NH], BF16, kind="ExternalOutput")

    n_tiles = NH // NT            # 16
    subs = NT // NSUB             # 4 sub-chunks per n-tile

    with tile.TileContext(nc) as tc, ExitStack() as ctx:
        const = ctx.enter_context(tc.tile_pool(name="const", bufs=1))
        xpool = ctx.enter_context(tc.tile_pool(name="x", bufs=1))
        fpool = ctx.enter_context(tc.tile_pool(name="f", bufs=1))
        stage = ctx.enter_context(tc.tile_pool(name="stage", bufs=4))
        spool = ctx.enter_context(tc.tile_pool(name="smax", bufs=1))
        opool = ctx.enter_context(tc.tile_pool(name="o", bufs=8))
        dram = ctx.enter_context(tc.tile_pool(name="dram", bufs=1, space="DRAM"))

        # identity first (Pool engine ops, needed by the first transpose),
        # then wft on the fast SP ring (needed by the first F^T matmul);
        # wbt (phase 2) can trail on Pool's SWDGE.
        ident = const.tile([128, 128], F32, tag="ident")
        make_identity(nc, ident[:])
        ident_r = const.tile([128, 128], F32R, tag="ident_r")
        nc.vector.tensor_copy(ident_r[:], ident[:])
        wft_sb = []
        wbt_sb = []
        for ci in range(2):
            t = const.tile([128, O], F32R, tag=f"wft{ci}")
            nc.gpsimd.dma_start(t[:], wft[ci])
            wft_sb.append(t)
            t = const.tile([128, C], F32R, tag=f"wbt{ci}")
            nc.gpsimd.dma_start(t[:], wbt[ci])
            wbt_sb.append(t)

        def one_iter(it: int):
            # resident x: 2 c-chunks; chunked loads so the PE can start after
            # the first 512 columns. ci=0 on the SP HWDGE ring, ci=1 on ACT.
            x_sb = []
            for ci in range(2):
                xt = xpool.tile([128, NH], F32R, tag=f"x_{ci}", name=f"x_{ci}")
                x_sb.append(xt)
            # load groups: 256, 256, 1024, ... (small first groups cut ramp)
            bounds = [0, 256, 512]
            while bounds[-1] < NH:
                bounds.append(min(NH, bounds[-1] + 1024))
            for g in range(len(bounds) - 1):
                lo, hi = bounds[g], bounds[g + 1]
                for ci in range(2):
                    # all x loads on the SP HWDGE ring (ci-interleaved): SP is
                    # otherwise idle in phase 1, while ACT/DVE/Pool sequencers
                    # must stay free for PSUM evacuations
                    nc.sync.dma_start(x_sb[ci][:, lo:hi], xs[ci, :, lo:hi])

            f_sb = fpool.tile([O, NH], F32R, tag="F")     # phase-3 rhs
            fT_sb = []                                     # F^T per n-tile

            # ---- phase 1: F^T, x^T, partial S^T (exact fp32) ----
            with tc.tile_pool(name=f"psS{it}", bufs=2, space="PSUM") as psS:
                # one bank per c-half: a PSUM bank supports a single open
                # matmul accumulation group; interleaving two regions' groups
                # in one bank corrupts both
                s_ps0 = psS.tile([128, O], F32, tag="S0")
                s_ps1 = psS.tile([128, O], F32, tag="S1")
                s_ps = [s_ps0, s_ps1]
                with tc.tile_pool(name=f"psF{it}", bufs=2, space="PSUM") as psF, \
                     tc.tile_pool(name=f"psT{it}", bufs=2, space="PSUM") as psT:
                    for nt in range(n_tiles):
                        n0 = nt * NT
                        # F^T for 4 sub-chunks -> one PSUM bank [128, 4, 50]
                        ft_ps = psF.tile([128, subs, O], F32, tag="ft_ps")
                        for s in range(subs):
                            sn0 = n0 + s * NSUB
                            for ci in range(2):
                                nc.tensor.matmul(
                                    ft_ps[:, s],
                                    x_sb[ci][:, sn0:sn0 + NSUB].bitcast(F32),
                                    wft_sb[ci][:].bitcast(F32),
                                    start=(ci == 0),
                                    stop=(ci == 1),
                                )
                        # x^T: 8 transposes -> 2 PSUM banks [128, 2, 256]
                        # fp32r transpose: bit-identical to the fp32 PE
                        # transpose (measured on HW) at 1.5 vs 2 cyc/row
                        xT_sb = []
                        for half in range(2):
                            tr_ps = psT.tile([128, 2, C], F32R, tag="tr")
                            for s2 in range(2):
                                sn0 = n0 + (half * 2 + s2) * NSUB
                                for ci in range(2):
                                    nc.tensor.transpose(
                                        tr_ps[:, s2,
                                              ci * 128:(ci + 1) * 128],
                                        x_sb[ci][:, sn0:sn0 + NSUB],
                                        ident_r[:],
                                    )
                            xT = stage.tile([128, 2, C], F32, tag="xT")
                            nc.vector.tensor_copy(xT[:], tr_ps[:].bitcast(F32))
                            xT_sb.append(xT)
                        fT = stage.tile([128, subs, O], F32, tag="fT")
                        nc.scalar.activation(fT[:], ft_ps[:], ActF.Copy)
                        fT_sb.append(fT)

                        # partial S^T += x^T_ci @ F^T   (ap=50, fp32)
                        for s in range(subs):
                            idx = nt * subs + s
                            for ci in range(2):
                                nc.tensor.matmul(
                                    s_ps[ci][:],
                                    xT_sb[s // 2][:, s % 2,
                                                  ci * 128:(ci + 1) * 128],
                                    fT[:, s],
                                    start=(idx == 0),
                                    stop=(idx == n_tiles * subs - 1),
                                )

                s_part = spool.tile([128, 2, O], F32, tag="s_part")
                nc.vector.tensor_copy(s_part[:, 0], s_ps0[:])
                nc.vector.tensor_copy(s_part[:, 1], s_ps1[:])

            # ---- phase 1.5: AllGather partial S^T; F in fp32r meanwhile ----
            use_ar = os.environ.get("CA_CC", "ag") == "ar"
            cc_in = dram.tile([128, 2, O], F32, tag="cc_in")
            cc_out = dram.tile([2, 128, 2, O], F32, tag="cc_out")
            cc_out_r = dram.tile([128, 2, O], F32, tag="cc_out_r")
            nc.sync.dma_start(cc_in[:], s_part[:])
            groups = [[0, 1], [2, 3], [4, 5], [6, 7]]
            if skip_cc:
                for r in range(2):
                    nc.sync.dma_start(cc_out[r], cc_in[:])
            elif use_ar:
                nc.gpsimd.collective_compute(
                    "AllReduce",
                    mybir.AluOpType.add,
                    replica_groups=groups,
                    ins=[cc_in.opt()],
                    outs=[cc_out_r.opt()],
                )
            else:
                nc.gpsimd.collective_compute(
                    "AllGather",
                    mybir.AluOpType.bypass,
                    replica_groups=groups,
                    ins=[cc_in.opt()],
                    outs=[cc_out.opt()],
                )
            s_gath = spool.tile([128, 2, 2, O], F32, tag="s_gath")
            if not use_ar:
                for r in range(2):
                    (nc.sync if r == 0 else nc.scalar).dma_start(
                        s_gath[:, r], cc_out[r]
                    )

            # F = W_f x in fp32r — fills the PE during the collective
            with tc.tile_pool(name=f"psF2{it}", bufs=3, space="PSUM") as psF2:
                for nt in range(n_tiles):
                    n0 = nt * NT
                    f_ps = psF2.tile([O, NT], F32, tag="f_ps")
                    for ci in range(2):
                        nc.tensor.matmul(
                            f_ps[:],
                            wft_sb[ci][:],
                            x_sb[ci][:, n0:n0 + NT],
                            start=(ci == 0),
                            stop=(ci == 1),
                        )
                    if nt % 2 == 0:
                        nc.scalar.activation(
                            f_sb[:, n0:n0 + NT], f_ps[:], ActF.Copy
                        )
                    else:
                        nc.vector.tensor_copy(f_sb[:, n0:n0 + NT], f_ps[:])

            # ---- phase 2: pair sum, transpose to S, softmax, M^T ----
            s_red = spool.tile([128, 2, O], F32, tag="s_red")
            if use_ar:
                nc.sync.dma_start(s_red[:], cc_out_r[:])
            else:
                nc.vector.tensor_add(s_red[:], s_gath[:, 0], s_gath[:, 1])
            with tc.tile_pool(name=f"psM{it}", bufs=2, space="PSUM") as psM:
                sf_ps = psM.tile([O, C], F32, tag="sf_ps")
                for ci in range(2):
                    nc.tensor.transpose(
                        sf_ps[:, ci * 128:(ci + 1) * 128],
                        s_red[:, ci], ident[:],
                    )

                mx = spool.tile([O, 1], F32, tag="mx")
                nc.vector.tensor_reduce(
                    mx[:], sf_ps[:], axis=mybir.AxisListType.X,
                    op=mybir.AluOpType.max,
                )
                nmx = spool.tile([O, 1], F32, tag="nmx")
                nc.vector.tensor_scalar_mul(nmx[:], mx[:], -1.0)
                p_exp = spool.tile([O, C], F32, tag="p_exp")
                ssum = spool.tile([O, 1], F32, tag="ssum")
                nc.scalar.activation(
                    p_exp[:], sf_ps[:], ActF.Exp, bias=nmx[:], accum_out=ssum[:]
                )
                rsum = spool.tile([O, 1], F32, tag="rsum")
                nc.vector.reciprocal(rsum[:], ssum[:])

                # M^T = (P @ W_beta^T) * rsum  — normalize folded into evac
                pT_ps = psM.tile([128, 2, O], F32, tag="pT_ps")
                for ci in range(2):
                    nc.tensor.transpose(
                        pT_ps[:, ci], p_exp[:, ci * 128:(ci + 1) * 128],
                        ident[:O, :O],
                    )
                pT_sb = spool.tile([128, 2, O], F32, tag="pT_sb")
                nc.vector.tensor_copy(pT_sb[:], pT_ps[:])
                m_ps = psM.tile([O, C], F32, tag="m_ps")
                for ci in range(2):
                    nc.tensor.matmul(
                        m_ps[:],
                        pT_sb[:, ci],
                        wbt_sb[ci][:].bitcast(F32),
                        start=(ci == 0),
                        stop=(ci == 1),
                    )
                mT_sb = spool.tile([O, C], F32R, tag="mT")
                nc.scalar.activation(
                    mT_sb[:], m_ps[:], ActF.Copy, scale=rsum[:]
                )

            if skip_phase3:
                return
            # ---- phase 3: out = M F + I x (PE accumulates the residual in
            # fp32r at full rate); evacuation is then a pure copy cycled over
            # ACT/DVE/Pool so no single engine gates the store stream.
            # 2 n-tiles per 512 KiB store, alternating SP/ACT HWDGE rings.
            with tc.tile_pool(name=f"psO{it}", bufs=6, space="PSUM") as psO:
                evac = 0
                for np2 in range(n_tiles // 2):
                    for d in range(2):
                        o_sb = opool.tile([128, 2 * NT], BF16, tag="o_sb")
                        for k in range(2):
                            nt = np2 * 2 + k
                            n0 = nt * NT
                            o_ps = psO.tile([128, NT], F32, tag="o_ps")
                            nc.tensor.matmul(
                                o_ps[:],
                                mT_sb[:, d * 128:(d + 1) * 128],
                                f_sb[:, n0:n0 + NT],
                                start=True,
                                stop=False,
                            )
                            nc.tensor.matmul(
                                o_ps[:],
                                ident_r[:],
                                x_sb[d][:, n0:n0 + NT],
                                start=False,
                                stop=True,
                            )
                            osl = o_sb[:, k * NT:(k + 1) * NT]
                            if evac % 2 == 0:
                                nc.scalar.activation(osl, o_ps[:], ActF.Copy)
                            else:
                                nc.vector.tensor_copy(osl, o_ps[:])
                            evac += 1
                        n0 = np2 * 2 * NT
                        nc.sync.dma_start(out[d, :, n0:n0 + 2 * NT], o_sb[:])

        for it in range(n_iters):
            one_iter(it)

    _split_multiwait(nc)
    return nc


def _get_nc(fast: bool = False, n_iters: int = 1):
    key = ("nc", n_iters, os.environ.get("CA_CC", "ag"))
    if key not in _CACHE:
        _CACHE[key] = _build_nc(fast, n_iters)
    return _CACHE[key]


def _make_in_maps(x, W_f, W_beta):
    xf = np.ascontiguousarray(x.reshape(B, C, N), dtype=np.float32)
    wft = np.ascontiguousarray(W_f.T.reshape(2, 128, O), dtype=np.float32)
    wbt = np.ascontiguousarray(W_beta.T.reshape(2, 128, C), dtype=np.float32)
    in_maps = []
    for c in range(NCORES):
        b, h = divmod(c, 2)
        shard = np.ascontiguousarray(
            xf[b, :, h * NH:(h + 1) * NH].reshape(2, 128, NH)
        )
        in_maps.append({"xs": shard, "wft": wft, "wbt": wbt})
    return in_maps


def kernel(x: np.ndarray, W_f: np.ndarray, W_beta: np.ndarray) -> np.ndarray:
    global last_results
    nc = _get_nc(False)

    in_maps = _make_in_maps(x, W_f, W_beta)
    res = run_bass_kernel_spmd(nc, in_maps, list(range(NCORES)))
    last_results = res

    outv = np.empty((B, C, N), dtype=np.float32)
    for c in range(NCORES):
        b, h = divmod(c, 2)
        outv[b, :, h * NH:(h + 1) * NH] = (
            res.results[c]["out"].astype(np.float32).reshape(C, NH)
        )
    return outv.reshape(B, C, 128, 128)
